# revision 41
# baseline (speedup 1.0000x reference)
"""Trainium2 Bass kernel for an MoE classification head.

Model (per reference):
    normed = LayerNorm(x)  (no affine; shared across gate+experts)
    gate   = softmax(normed * g_g + b_g) @ gate_w + gate_b)      [B, E]
    h_e    = GELU((normed * g_e + b_e) @ w1_e + b1_e)            [E, B, H]
    out    = sum_e gate[:, e] * (h_e @ w2_e + b2_e)              [B, C]

Strategy: data-parallel over 8 NeuronCores (batch sharded 2048 rows/core,
all parameters replicated).  The per-expert LayerNorm affine folds into
w1/b1 on the host (normed*g+b) @ w1 == normed @ (g*w1) + b@w1, same for
the gate.  Matmul operands are bf16 (PE runs 1 row/cycle, same as fp32r,
but bf16 halves weight DMA + SBUF and transposes at 1 cyc/row); PSUM
accumulation stays fp32, LN stats / softmax / final gated sum stay fp32.

Schedule: x arrives as bf16 (host-converted; normed is bf16-rounded for
the matmuls anyway, so quantizing before LN only adds sqrt(2)x that
rounding).  All DMA shares one queue in exact consumption order: chunk-0
x tiles, small weights, w1 for expert 0 in H-quarter pieces, chunk-1's x
tiles, then the rest of w1 (so the chunk-1 transposes never wait behind
the 16MB w1 bulk).  The batch is processed in 512-row chunks: per chunk,
4 experts of 16 [8x mm1 chain + GELU] h-steps, then the expert's 16
narrow mm2s BATCHED back-to-back into one PSUM accumulate chain (HW
measurement: an isolated [128,3]-stationary matmul costs ~370ns plus a
~127ns weight-shadow penalty on the following mm1; batched chains stream
at the full 216ns moving rate).  Next-chunk transposes / gate softmax /
LN prep run at expert boundaries where the PE stream is already broken;
per-expert gating (DVE) is deferred one expert so it overlaps matmuls.
~40 junk transposes at kernel start keep the HAM clock warm through the
initial DMA fill.  Measured on HW (neuron-profile, 8 cores): 543us/core
at ~2.38GHz PE clock (~96% PE-busy; the mm1 moving-row floor alone is
441us).  Note the chip clock varies run-to-run (~2.0-2.38GHz), +-18% on
total time; baseline before this restructure profiled at 706us.
"""

import os

import numpy as np
import ml_dtypes

import concourse.bacc as bacc
import concourse.mybir as mybir
from concourse import tile, masks
from concourse.bass_utils import run_bass_kernel_spmd

F32 = mybir.dt.float32
BF16 = mybir.dt.bfloat16
AF = mybir.ActivationFunctionType
ALU = mybir.AluOpType

N_CORES = 8
B, D, H, E, C = 16384, 1024, 2048, 4, 3
BS = B // N_CORES       # 2048 rows per core
NT = BS // 128          # 16 batch tiles of 128 rows
KC = D // 128           # 8 contraction chunks over D
NBC = BS // 512         # 4 batch chunks of 512 (matmul moving dim)
TPC = NT // NBC         # 4 tiles per batch chunk
NHC = H // 128          # 16 H chunks
EPS = 1e-5
GELU_FUNC = AF.Gelu  # sim harness may swap (CoreSim lacks Gelu)

_NC_CACHE = {}
PHASE_LEVEL = int(os.environ.get("PHASE_LEVEL", "99"))  # build truncation knob


def _build_nc():
    nc = bacc.Bacc("TRN2", target_bir_lowering=False, debug=False,
                   enable_asserts=True, num_devices=N_CORES)
    x = nc.dram_tensor("x", [BS, D], BF16, kind="ExternalInput")
    gw = nc.dram_tensor("gw", [128, KC, E], BF16, kind="ExternalInput")
    gb = nc.dram_tensor("gb", [128, E], BF16, kind="ExternalInput")
    w1 = nc.dram_tensor("w1", [E, 128, KC, H], BF16, kind="ExternalInput")
    b1 = nc.dram_tensor("b1", [128, E, NHC], F32, kind="ExternalInput")
    w2 = nc.dram_tensor("w2", [128, E, NHC * C], BF16, kind="ExternalInput")
    b2 = nc.dram_tensor("b2", [C, E], F32, kind="ExternalInput")
    # three partial-sum row groups (mm2 PSUM slices at bases 0/32/64);
    # the host sums them -- cheaper than folding across partitions on-chip
    y = nc.dram_tensor("y", [3 * C, BS], F32, kind="ExternalOutput")

    with tile.TileContext(nc) as tc:
        with (
            tc.tile_pool(name="pers", bufs=1) as pers,
            tc.tile_pool(name="xp", bufs=4) as xp,
            tc.tile_pool(name="st", bufs=2) as st,
            tc.tile_pool(name="hp", bufs=2) as hp,
            tc.tile_pool(name="php", bufs=6, space="PSUM") as php,
            tc.tile_pool(name="plp", bufs=2, space="PSUM") as plp,
        ):
            # ---- persistent tiles ----
            normedT = pers.tile([128, KC, BS], BF16)   # normalized x, transposed
            w1sb = pers.tile([128, E, KC, H], BF16)    # all experts' w1
            # gate weights: expert e x3 rows at partitions 32e..32e+2 (bf16:
            # the values are already bf16-rounded by the transpose path)
            gwT3 = pers.tile([128, BS], BF16)
            # gated-output accumulator: three row-groups at partition bases
            # 0/32/64 matching the mm2 PSUM slices; folded to rows 0-2 at
            # the end of each chunk
            accT3 = pers.tile([128, BS], F32)
            identb = pers.tile([128, 128], BF16)
            gwsb = pers.tile([128, KC, E], BF16)
            gbsb = pers.tile([128, E], BF16)
            b1sb = pers.tile([128, E, NHC], F32)
            w2sb = pers.tile([128, E, NHC * C], BF16)
            b2sb = pers.tile([C, E], F32)
            epst = pers.tile([128, 1], F32)
            nc.vector.memset(epst[:], EPS)

            masks.make_identity(nc, identb[:])

            # warm the PE HAM clock gate during the startup DMA fill: the
            # PE is otherwise idle for ~8us, and its first ~3.4us of real
            # work would run at the cold 1.2GHz half-clock.  ~100 no-reader
            # transposes keep the activity window busy until real work lands.
            if PHASE_LEVEL >= 2:
                junk = php.tile([128, 128], BF16, tag="mm")
                for _ in range(40):
                    nc.tensor.transpose(junk[:], identb[:], identb[:])

            nrms = {}
            gwrs = {}

            xts = {}

            def prep_dma(bc):
                """Issue chunk bc's x-tile DMAs (placement in the single
                FIFO DMA queue decides when the tiles land)."""
                xts[bc] = []
                for t in range(TPC):
                    ti = bc * TPC + t
                    bsl = slice(ti * 128, (ti + 1) * 128)
                    xt = xp.tile([128, D], BF16, tag="xt", bufs=4)
                    nc.sync.dma_start(xt[:], x[bsl, :])
                    xts[bc].append(xt)

            def prep(bc):
                """LN for chunk bc: bn_stats + rsqrt + fused normalize."""
                if bc not in xts:
                    prep_dma(bc)
                tiles = []
                for t in range(TPC):
                    xt = xts[bc][t]
                    if PHASE_LEVEL < 1:
                        tiles.append(None)
                        continue
                    stats = st.tile([128, 2, 6], F32, tag="stats")
                    nc.vector.bn_stats(stats[:, 0], xt[:, 0:512])
                    nc.vector.bn_stats(stats[:, 1], xt[:, 512:1024])
                    mv = st.tile([128, 2], F32, tag="mv")
                    nc.vector.bn_aggr(mv[:], stats[:])
                    sd = st.tile([128, 1], F32, tag="sd")
                    nc.scalar.activation(sd[:], mv[:, 1:2], AF.Sqrt, bias=epst[:])
                    rs = st.tile([128, 1], F32, tag="rs")
                    nc.vector.reciprocal(rs[:], sd[:])
                    nrm = xp.tile([128, D], BF16, tag="nrm", bufs=4)
                    nc.vector.tensor_scalar(nrm[:], xt[:], mv[:, 0:1], rs[:],
                                            ALU.subtract, ALU.mult)
                    tiles.append(nrm)
                nrms[bc] = tiles

            def tg_tile_T(bc, t):
                """Transpose one tile of chunk bc into normedT."""
                ti = bc * TPC + t
                bsl = slice(ti * 128, (ti + 1) * 128)
                nrm = nrms[bc][t]
                if nrm is None or PHASE_LEVEL < 2:
                    return
                # 8 transposes share one PSUM bank; one strided copy drains it
                pt8 = php.tile([128, KC, 128], BF16, tag="mm")
                for kc in range(KC):
                    nc.tensor.transpose(pt8[:, kc], nrm[:, kc * 128:(kc + 1) * 128],
                                        identb[:])
                nc.vector.tensor_copy(normedT[:, :, bsl], pt8[:])

            def tg_gates(bc):
                """Gate matmuls + softmax for all tiles of chunk bc; the Exps
                are consecutive on the scalar queue (one table switch)."""
                gwrs[bc] = []
                if PHASE_LEVEL < 3:
                    return
                for t in range(TPC):
                    ti = bc * TPC + t
                    bsl = slice(ti * 128, (ti + 1) * 128)
                    pg = php.tile([128, E], F32, tag="mm")
                    for kc in range(KC):
                        nc.tensor.matmul(pg[:], normedT[:, kc, bsl], gwsb[:, kc, :],
                                         start=(kc == 0), stop=(kc == KC - 1))
                    # gate bias: DVE add (gb host-replicated to 128 rows)
                    # instead of a ones-stationary PE matmul
                    nc.vector.tensor_tensor(pg[:], pg[:], gbsb[:], ALU.add)

                    # softmax without max-subtraction: |logits| <~ 8 so exp is safe
                    exg = xp.tile([128, E], F32, tag="exg", bufs=2)
                    nc.scalar.activation(exg[:], pg[:], AF.Exp)
                    sme = st.tile([128, 1], F32, tag="sme")
                    nc.vector.reduce_sum(sme[:], exg[:], axis=mybir.AxisListType.X)
                    rinv = st.tile([128, 1], F32, tag="rinv")
                    nc.vector.reciprocal(rinv[:], sme[:])
                    if PHASE_LEVEL < 4:
                        continue
                    gwr = xp.tile([128, 128], BF16, tag="gwr", bufs=4)
                    nc.vector.memset(gwr[:], 0.0)
                    gwr4 = gwr[:].rearrange("p (e q) -> p e q", q=32)
                    for j in range(C):
                        nc.vector.tensor_scalar_mul(gwr4[:, :, j], exg[:], rinv[:])
                    gwrs[bc].append(gwr)

            def transpose_gate(bc):
                for t in range(TPC):
                    tg_tile_T(bc, t)
                tg_gates(bc)

            def gate_transpose(bc):
                """Deferred: transpose chunk bc's gate weights into gwT3.
                Runs early in the expert phase so the PE never waits on the
                softmax chain."""
                for t, gwr in enumerate(gwrs[bc]):
                    ti = bc * TPC + t
                    bsl = slice(ti * 128, (ti + 1) * 128)
                    pgt = php.tile([128, 128], BF16, tag="mm")
                    nc.tensor.transpose(pgt[:], gwr[:], identb[:])
                    nc.vector.tensor_copy(gwT3[:, bsl], pgt[:])

            pend_gate = []  # deferred (bc, e, pl) gating work

            def queue_gating(bc, e, pl):
                pend_gate.append((bc, e, pl))

            def flush_gating():
                """Emit gating for completed experts; runs while the PE is
                deep in the next expert's matmuls so the DVE is free at
                chunk boundaries.  pl holds three partial sums at partition
                bases 0/32/64; gate+accumulate each slice in place (DVE
                lanes must match, so the gate row is DMA-replicated to the
                three bases), then fold the three accumulator row-groups
                down to rows 0-2 once per chunk before the y DMA."""
                while pend_gate:
                    gbc, ge, gpl = pend_gate.pop(0)
                    gsl = slice(gbc * 512, (gbc + 1) * 512)
                    gwb = hp.tile([128, 512], BF16, tag="gwb", bufs=1)
                    for j in range(3):
                        nc.sync.dma_start(gwb[32 * j:32 * j + C, :],
                                          gwT3[32 * ge:32 * ge + C, gsl])
                    lt = hp.tile([128, 512], F32, tag="lt", bufs=1)
                    # b2 is added on slice 0 only (once per expert)
                    nc.vector.scalar_tensor_tensor(lt[0:C], gpl[0:C, :],
                                                   b2sb[:, ge:ge + 1],
                                                   gwb[0:C], ALU.add, ALU.mult)
                    for j in (1, 2):
                        nc.vector.tensor_tensor(
                            lt[32 * j:32 * j + C], gpl[32 * j:32 * j + C, :],
                            gwb[32 * j:32 * j + C], ALU.mult)
                    for j in range(3):
                        nc.vector.tensor_add(
                            accT3[32 * j:32 * j + C, gsl],
                            accT3[32 * j:32 * j + C, gsl],
                            lt[32 * j:32 * j + C])
                    if ge == E - 1:
                        for j in range(3):
                            nc.sync.dma_start(
                                y[C * j:C * j + C, gsl],
                                accT3[32 * j:32 * j + C, gsl])

            def experts(bc):
                csl = slice(bc * 512, (bc + 1) * 512)
                for e in range(E):
                    # gate-weight transposes at the e0/e1 boundary: the PE
                    # stream is already broken there by e0's mm2 batch, and
                    # by then the softmax chain (ACT table switch + Exp +
                    # DVE) is long finished -- at the chunk boundary it is
                    # not, and the PE would stall ~3us waiting for it
                    if e == 1 and PHASE_LEVEL >= 4:
                        gate_transpose(bc)
                    hts = []
                    for hg in range(NHC):
                        ph = php.tile([128, 512], F32, tag="mm")
                        for kc in range(KC):
                            nc.tensor.matmul(
                                ph[:],
                                w1sb[:, e, kc, hg * 128:(hg + 1) * 128],
                                normedT[:, kc, csl],
                                start=(kc == 0), stop=(kc == KC - 1))
                        if hg == 1 and PHASE_LEVEL >= 6:
                            flush_gating()
                        hT = hp.tile([128, 512], BF16, tag="hT", bufs=14)
                        nc.scalar.activation(hT[:], ph[:], GELU_FUNC,
                                             bias=b1sb[:, e, hg:hg + 1])
                        hts.append(hT)
                    # mm2 batch: 16 narrow matmuls back-to-back, accumulate
                    # slices spread over THREE PE array column tiles (out
                    # base partitions 0/32/64).  Measured on HW: narrow
                    # matmuls on different column tiles overlap in the
                    # array, 2.6x faster than a single-slice chain (and
                    # isolated narrow matmuls interleaved into the mm1
                    # stream are worse still).  The three partial sums are
                    # combined during the (deferred) gating pass.
                    pl = plp.tile([128, 512], F32, tag="pl")
                    for hg in range(NHC):
                        j = 32 * (hg % 3)
                        nc.tensor.matmul(
                            pl[j:j + C, :], w2sb[:, e, hg * C:(hg + 1) * C],
                            hts[hg][:],
                            start=(hg < 3), stop=(hg >= NHC - 3))
                    if PHASE_LEVEL >= 6:
                        queue_gating(bc, e, pl)
                    # next chunk's transposes/gate/LN at the expert boundary:
                    # the PE stream is already broken here by the mm2 batch,
                    # so the transpose mode-switch penalty is amortized
                    if bc + 1 < NBC:
                        # tiles front-loaded (0,1@e0 1@e1 1@e2) so tile 3's
                        # normedT copy lands well before tg_gates reads it
                        for t in ([0, 1], [2], [3], [])[e]:
                            tg_tile_T(bc + 1, t)
                        if e == E - 1:
                            tg_gates(bc + 1)
                            if bc + 2 < NBC:
                                prep(bc + 2)

            prep_dma(0)  # first: chunk-0 x tiles lead the DMA queue
            prep(0)      # chunk-0 LN chain leads the DVE queue
            # memsets after prep so the DVE reaches LN stats immediately
            nc.vector.memset(accT3[:], 0.0)
            nc.sync.dma_start(gwsb[:], gw[:])
            nc.sync.dma_start(gbsb[:], gb[:])
            nc.sync.dma_start(b2sb[:], b2[:])
            nc.sync.dma_start(b1sb[:], b1[:])
            nc.sync.dma_start(w2sb[:], w2[:])
            # w1 preload on the SAME sync queue, behind chunk-0's x tiles:
            # single-queue FIFO gives the exact consumption order.
            # H-quarter pieces let expert 0's first mm1 groups start after
            # 1MB instead of 4MB.  Chunk-1's x tiles slot in right after
            # expert 0's w1 so the chunk-1 transposes (which now run at
            # expert boundaries of chunk 0) never wait on the 16MB w1 bulk.
            HQ = H // 4
            for e in range(E):
                for q in range(4):
                    nc.sync.dma_start(w1sb[:, e, :, q * HQ:(q + 1) * HQ],
                                      w1[e, :, :, q * HQ:(q + 1) * HQ])
                if e == 0:
                    prep_dma(1)
            if PHASE_LEVEL >= 5:
                transpose_gate(0)
                prep(1)
                for bc in range(NBC):
                    experts(bc)
                flush_gating()
            else:
                for bc in range(NBC):
                    transpose_gate(bc)
                    if bc + 1 < NBC:
                        prep(bc + 1)
                nc.sync.dma_start(y[0:C, :], accT3[0:C, :])

    nc.finalize()
    return nc


def _fold_inputs(inputs):
    x = np.asarray(inputs["x"], np.float32)
    gg = np.asarray(inputs["gate_ln_g"], np.float32)
    gbeta = np.asarray(inputs["gate_ln_b"], np.float32)
    gw_ = np.asarray(inputs["gate_w"], np.float32)
    gbias = np.asarray(inputs["gate_b"], np.float32)
    eg = np.asarray(inputs["ex_ln_g"], np.float32)
    eb = np.asarray(inputs["ex_ln_b"], np.float32)
    w1_ = np.asarray(inputs["ex_w1"], np.float32)
    b1_ = np.asarray(inputs["ex_b1"], np.float32)
    w2_ = np.asarray(inputs["ex_w2"], np.float32)
    b2_ = np.asarray(inputs["ex_b2"], np.float32)

    # fold the (shared-normalize, per-head affine) LayerNorms into the
    # following linear layers: (n*g+b) @ W == n @ (g[:,None]*W) + b@W
    gwf = (gg[:, None] * gw_).astype(np.float32)                    # [D, E]
    gbf = (gbias + gbeta @ gw_).astype(np.float32)                  # [E]
    w1f = (eg[:, :, None] * w1_).astype(np.float32)                 # [E, D, H]
    b1f = (b1_ + np.einsum("ed,edh->eh", eb, w1_)).astype(np.float32)

    bf16 = ml_dtypes.bfloat16
    gw_dev = np.ascontiguousarray(
        gwf.reshape(KC, 128, E).transpose(1, 0, 2)).astype(bf16)
    gb_dev = np.ascontiguousarray(np.tile(gbf.reshape(1, E), (128, 1))).astype(bf16)
    w1_dev = np.ascontiguousarray(
        w1f.reshape(E, KC, 128, H).transpose(0, 2, 1, 3)).astype(bf16)
    b1_dev = np.ascontiguousarray(
        b1f.reshape(E, NHC, 128).transpose(2, 0, 1))                # [128, E, NHC]
    w2_dev = np.ascontiguousarray(
        w2_.reshape(E, NHC, 128, C).transpose(2, 0, 1, 3).reshape(
            128, E, NHC * C)).astype(bf16)
    b2_dev = np.ascontiguousarray(b2_.T)
    weights = dict(gw=gw_dev, gb=gb_dev, w1=w1_dev, b1=b1_dev,
                   w2=w2_dev, b2=b2_dev)
    return x, weights


def _get_nc():
    if "nc" not in _NC_CACHE:
        _NC_CACHE["nc"] = _build_nc()
    return _NC_CACHE["nc"]


def _in_maps(inputs):
    x, weights = _fold_inputs(inputs)
    maps = []
    for c in range(N_CORES):
        m = dict(weights)
        m["x"] = np.ascontiguousarray(x[c * BS:(c + 1) * BS]).astype(
            ml_dtypes.bfloat16)
        maps.append(m)
    return maps


def kernel(**inputs) -> np.ndarray:
    nc = _get_nc()
    maps = _in_maps(inputs)
    try:
        res = run_bass_kernel_spmd(nc, maps, list(range(N_CORES))).results
    except Exception:  # transient device error: retry once
        res = run_bass_kernel_spmd(nc, maps, list(range(N_CORES))).results
    out = np.empty((B, C), np.float32)
    for c in range(N_CORES):
        yc = res[c]["y"]
        out[c * BS:(c + 1) * BS] = (yc[0:C] + yc[C:2 * C] + yc[2 * C:3 * C]).T
    return out



# revision 43
# speedup vs baseline: 1.0399x; 1.0399x over previous
"""Trainium2 Bass kernel for an MoE classification head.

Model (per reference):
    normed = LayerNorm(x)  (no affine; shared across gate+experts)
    gate   = softmax(normed * g_g + b_g) @ gate_w + gate_b)      [B, E]
    h_e    = GELU((normed * g_e + b_e) @ w1_e + b1_e)            [E, B, H]
    out    = sum_e gate[:, e] * (h_e @ w2_e + b2_e)              [B, C]

Strategy: data-parallel over 8 NeuronCores (batch sharded 2048 rows/core,
all parameters replicated).  The per-expert LayerNorm affine folds into
w1/b1 on the host (normed*g+b) @ w1 == normed @ (g*w1) + b@w1, same for
the gate.  Matmul operands are bf16 (PE runs 1 row/cycle, same as fp32r,
but bf16 halves weight DMA + SBUF and transposes at 1 cyc/row); PSUM
accumulation stays fp32, LN stats / softmax / final gated sum stay fp32.

Schedule: x arrives as bf16 (host-converted; normed is bf16-rounded for
the matmuls anyway, so quantizing before LN only adds sqrt(2)x that
rounding).  All DMA shares one queue in exact consumption order: chunk-0
x tiles, small weights, w1 for expert 0 in H-quarter pieces, chunk-1's x
tiles, then the rest of w1 (so the chunk-1 transposes never wait behind
the 16MB w1 bulk).  The batch is processed in 512-row chunks: per chunk,
4 experts of 16 [8x mm1 chain + GELU] h-steps, then the expert's 16
narrow mm2s BATCHED back-to-back into one PSUM accumulate chain (HW
measurement: an isolated [128,3]-stationary matmul costs ~370ns plus a
~127ns weight-shadow penalty on the following mm1; batched chains stream
at the full 216ns moving rate).  Next-chunk transposes / gate softmax /
LN prep run at expert boundaries where the PE stream is already broken;
per-expert gating (DVE) is deferred one expert so it overlaps matmuls.
~40 junk transposes at kernel start keep the HAM clock warm through the
initial DMA fill.  The mm2 accumulate slices are spread over three PE
array column tiles (out bases 0/32/64) -- measured 2.6x faster than a
single-slice chain -- and the three partial outputs ship to y[9, BS]
for a host-side sum.  Measured on HW (neuron-profile, 8 cores):
~525-535us/core at ~2.38GHz (~96% PE-busy; the mm1 moving-row floor
alone is 441us).  The chip clock varies run-to-run (~2.0-2.38GHz),
+-18% on total time; baseline before this restructure profiled 706us.
"""

import os

import numpy as np
import ml_dtypes

import concourse.bacc as bacc
import concourse.mybir as mybir
from concourse import tile, masks
from concourse.bass_utils import run_bass_kernel_spmd

F32 = mybir.dt.float32
BF16 = mybir.dt.bfloat16
AF = mybir.ActivationFunctionType
ALU = mybir.AluOpType

N_CORES = 8
B, D, H, E, C = 16384, 1024, 2048, 4, 3
BS = B // N_CORES       # 2048 rows per core
NT = BS // 128          # 16 batch tiles of 128 rows
KC = D // 128           # 8 contraction chunks over D
NBC = BS // 512         # 4 batch chunks of 512 (matmul moving dim)
TPC = NT // NBC         # 4 tiles per batch chunk
NHC = H // 128          # 16 H chunks
EPS = 1e-5
GELU_FUNC = AF.Gelu  # sim harness may swap (CoreSim lacks Gelu)

_NC_CACHE = {}
PHASE_LEVEL = int(os.environ.get("PHASE_LEVEL", "99"))  # build truncation knob


def _build_nc():
    nc = bacc.Bacc("TRN2", target_bir_lowering=False, debug=False,
                   enable_asserts=True, num_devices=N_CORES)
    x = nc.dram_tensor("x", [BS, D], BF16, kind="ExternalInput")
    gw = nc.dram_tensor("gw", [128, KC, E], BF16, kind="ExternalInput")
    gb = nc.dram_tensor("gb", [128, E], BF16, kind="ExternalInput")
    w1 = nc.dram_tensor("w1", [E, 128, KC, H], BF16, kind="ExternalInput")
    b1 = nc.dram_tensor("b1", [128, E, NHC], F32, kind="ExternalInput")
    w2 = nc.dram_tensor("w2", [128, E, NHC * C], BF16, kind="ExternalInput")
    b2 = nc.dram_tensor("b2", [C, E], F32, kind="ExternalInput")
    # three partial-sum row groups (mm2 PSUM slices at bases 0/32/64);
    # the host sums them -- cheaper than folding across partitions on-chip
    y = nc.dram_tensor("y", [3 * C, BS], F32, kind="ExternalOutput")

    with tile.TileContext(nc) as tc:
        with (
            tc.tile_pool(name="pers", bufs=1) as pers,
            tc.tile_pool(name="xp", bufs=4) as xp,
            tc.tile_pool(name="st", bufs=2) as st,
            tc.tile_pool(name="hp", bufs=2) as hp,
            tc.tile_pool(name="php", bufs=6, space="PSUM") as php,
            tc.tile_pool(name="plp", bufs=2, space="PSUM") as plp,
        ):
            # ---- persistent tiles ----
            normedT = pers.tile([128, KC, BS], BF16)   # normalized x, transposed
            w1sb = pers.tile([128, E, KC, H], BF16)    # all experts' w1
            # gate weights: expert e x3 rows at partitions 32e..32e+2 (bf16:
            # the values are already bf16-rounded by the transpose path)
            gwT3 = pers.tile([128, BS], BF16)
            # gated-output accumulator: three row-groups at partition bases
            # 0/32/64 matching the mm2 PSUM slices; folded to rows 0-2 at
            # the end of each chunk
            accT3 = pers.tile([128, BS], F32)
            identb = pers.tile([128, 128], BF16)
            gwsb = pers.tile([128, KC, E], BF16)
            gbsb = pers.tile([128, E], BF16)
            b1sb = pers.tile([128, E, NHC], F32)
            w2sb = pers.tile([128, E, NHC * C], BF16)
            b2sb = pers.tile([C, E], F32)
            epst = pers.tile([128, 1], F32)
            nc.vector.memset(epst[:], EPS)

            masks.make_identity(nc, identb[:])

            # warm the PE HAM clock gate during the startup DMA fill: the
            # PE is otherwise idle for ~8us, and its first ~3.4us of real
            # work would run at the cold 1.2GHz half-clock.  ~100 no-reader
            # transposes keep the activity window busy until real work lands.
            if PHASE_LEVEL >= 2:
                junk = php.tile([128, 128], BF16, tag="mm")
                for _ in range(40):
                    nc.tensor.transpose(junk[:], identb[:], identb[:])

            nrms = {}
            gwrs = {}

            xts = {}

            def prep_dma(bc):
                """Issue chunk bc's x-tile DMAs (placement in the single
                FIFO DMA queue decides when the tiles land)."""
                xts[bc] = []
                for t in range(TPC):
                    ti = bc * TPC + t
                    bsl = slice(ti * 128, (ti + 1) * 128)
                    xt = xp.tile([128, D], BF16, tag="xt", bufs=4)
                    nc.sync.dma_start(xt[:], x[bsl, :])
                    xts[bc].append(xt)

            def prep(bc):
                """LN for chunk bc: bn_stats + rsqrt + fused normalize."""
                if bc not in xts:
                    prep_dma(bc)
                tiles = []
                for t in range(TPC):
                    xt = xts[bc][t]
                    if PHASE_LEVEL < 1:
                        tiles.append(None)
                        continue
                    stats = st.tile([128, 2, 6], F32, tag="stats")
                    nc.vector.bn_stats(stats[:, 0], xt[:, 0:512])
                    nc.vector.bn_stats(stats[:, 1], xt[:, 512:1024])
                    mv = st.tile([128, 2], F32, tag="mv")
                    nc.vector.bn_aggr(mv[:], stats[:])
                    sd = st.tile([128, 1], F32, tag="sd")
                    nc.scalar.activation(sd[:], mv[:, 1:2], AF.Sqrt, bias=epst[:])
                    rs = st.tile([128, 1], F32, tag="rs")
                    nc.vector.reciprocal(rs[:], sd[:])
                    nrm = xp.tile([128, D], BF16, tag="nrm", bufs=3)
                    nc.vector.tensor_scalar(nrm[:], xt[:], mv[:, 0:1], rs[:],
                                            ALU.subtract, ALU.mult)
                    tiles.append(nrm)
                nrms[bc] = tiles

            def tg_tile_T(bc, t):
                """Transpose one tile of chunk bc into normedT."""
                ti = bc * TPC + t
                bsl = slice(ti * 128, (ti + 1) * 128)
                nrm = nrms[bc][t]
                if nrm is None or PHASE_LEVEL < 2:
                    return
                # 8 transposes share one PSUM bank; one strided copy drains it
                pt8 = php.tile([128, KC, 128], BF16, tag="mm")
                for kc in range(KC):
                    nc.tensor.transpose(pt8[:, kc], nrm[:, kc * 128:(kc + 1) * 128],
                                        identb[:])
                nc.vector.tensor_copy(normedT[:, :, bsl], pt8[:])

            def tg_gates(bc):
                """Gate matmuls + softmax for all tiles of chunk bc; the Exps
                are consecutive on the scalar queue (one table switch)."""
                gwrs[bc] = []
                if PHASE_LEVEL < 3:
                    return
                for t in range(TPC):
                    ti = bc * TPC + t
                    bsl = slice(ti * 128, (ti + 1) * 128)
                    pg = php.tile([128, E], F32, tag="mm")
                    for kc in range(KC):
                        nc.tensor.matmul(pg[:], normedT[:, kc, bsl], gwsb[:, kc, :],
                                         start=(kc == 0), stop=(kc == KC - 1))
                    # gate bias: DVE add (gb host-replicated to 128 rows)
                    # instead of a ones-stationary PE matmul
                    nc.vector.tensor_tensor(pg[:], pg[:], gbsb[:], ALU.add)

                    # softmax without max-subtraction: |logits| <~ 8 so exp is safe
                    exg = xp.tile([128, E], F32, tag="exg", bufs=2)
                    nc.scalar.activation(exg[:], pg[:], AF.Exp)
                    sme = st.tile([128, 1], F32, tag="sme")
                    nc.vector.reduce_sum(sme[:], exg[:], axis=mybir.AxisListType.X)
                    rinv = st.tile([128, 1], F32, tag="rinv")
                    nc.vector.reciprocal(rinv[:], sme[:])
                    if PHASE_LEVEL < 4:
                        continue
                    gwr = xp.tile([128, 128], BF16, tag="gwr", bufs=4)
                    nc.vector.memset(gwr[:], 0.0)
                    gwr4 = gwr[:].rearrange("p (e q) -> p e q", q=32)
                    for j in range(C):
                        nc.vector.tensor_scalar_mul(gwr4[:, :, j], exg[:], rinv[:])
                    gwrs[bc].append(gwr)

            def transpose_gate(bc):
                for t in range(TPC):
                    tg_tile_T(bc, t)
                tg_gates(bc)

            def gate_transpose(bc):
                """Deferred: transpose chunk bc's gate weights into gwT3.
                Runs early in the expert phase so the PE never waits on the
                softmax chain."""
                for t, gwr in enumerate(gwrs[bc]):
                    ti = bc * TPC + t
                    bsl = slice(ti * 128, (ti + 1) * 128)
                    pgt = php.tile([128, 128], BF16, tag="mm")
                    nc.tensor.transpose(pgt[:], gwr[:], identb[:])
                    nc.vector.tensor_copy(gwT3[:, bsl], pgt[:])

            pend_gate = []  # deferred (bc, e, pl) gating work

            def queue_gating(bc, e, pl):
                pend_gate.append((bc, e, pl))

            def flush_gating():
                """Emit gating for completed experts; runs while the PE is
                deep in the next expert's matmuls so the DVE is free at
                chunk boundaries.  pl holds three partial sums at partition
                bases 0/32/64; gate+accumulate each slice in place (DVE
                lanes must match, so the gate row is DMA-replicated to the
                three bases), then fold the three accumulator row-groups
                down to rows 0-2 once per chunk before the y DMA."""
                while pend_gate:
                    gbc, ge, gpl = pend_gate.pop(0)
                    gsl = slice(gbc * 512, (gbc + 1) * 512)
                    gwb = hp.tile([128, 512], BF16, tag="gwb", bufs=1)
                    for j in range(3):
                        nc.sync.dma_start(gwb[32 * j:32 * j + C, :],
                                          gwT3[32 * ge:32 * ge + C, gsl])
                    lt = hp.tile([128, 512], F32, tag="lt", bufs=1)
                    # b2 is added on slice 0 only (once per expert)
                    nc.vector.scalar_tensor_tensor(lt[0:C], gpl[0:C, :],
                                                   b2sb[:, ge:ge + 1],
                                                   gwb[0:C], ALU.add, ALU.mult)
                    for j in (1, 2):
                        nc.vector.tensor_tensor(
                            lt[32 * j:32 * j + C], gpl[32 * j:32 * j + C, :],
                            gwb[32 * j:32 * j + C], ALU.mult)
                    for j in range(3):
                        nc.vector.tensor_add(
                            accT3[32 * j:32 * j + C, gsl],
                            accT3[32 * j:32 * j + C, gsl],
                            lt[32 * j:32 * j + C])
                    if ge == E - 1:
                        for j in range(3):
                            nc.sync.dma_start(
                                y[C * j:C * j + C, gsl],
                                accT3[32 * j:32 * j + C, gsl])

            def experts(bc):
                csl = slice(bc * 512, (bc + 1) * 512)
                for e in range(E):
                    # gate-weight transposes at the e0/e1 boundary: the PE
                    # stream is already broken there by e0's mm2 batch, and
                    # by then the softmax chain (ACT table switch + Exp +
                    # DVE) is long finished -- at the chunk boundary it is
                    # not, and the PE would stall ~3us waiting for it
                    if e == 1 and PHASE_LEVEL >= 4:
                        gate_transpose(bc)
                    hts = []
                    for hg in range(NHC):
                        ph = php.tile([128, 512], F32, tag="mm")
                        for kc in range(KC):
                            nc.tensor.matmul(
                                ph[:],
                                w1sb[:, e, kc, hg * 128:(hg + 1) * 128],
                                normedT[:, kc, csl],
                                start=(kc == 0), stop=(kc == KC - 1))
                        if hg == 1 and PHASE_LEVEL >= 6:
                            flush_gating()
                        hT = hp.tile([128, 512], BF16, tag="hT", bufs=16)
                        nc.scalar.activation(hT[:], ph[:], GELU_FUNC,
                                             bias=b1sb[:, e, hg:hg + 1])
                        hts.append(hT)
                    # mm2 batch: 16 narrow matmuls back-to-back, accumulate
                    # slices spread over THREE PE array column tiles (out
                    # base partitions 0/32/64).  Measured on HW: narrow
                    # matmuls on different column tiles overlap in the
                    # array, 2.6x faster than a single-slice chain (and
                    # isolated narrow matmuls interleaved into the mm1
                    # stream are worse still).  The three partial sums are
                    # combined during the (deferred) gating pass.
                    pl = plp.tile([128, 512], F32, tag="pl")
                    for hg in range(NHC):
                        j = 32 * (hg % 3)
                        nc.tensor.matmul(
                            pl[j:j + C, :], w2sb[:, e, hg * C:(hg + 1) * C],
                            hts[hg][:],
                            start=(hg < 3), stop=(hg >= NHC - 3))
                    if PHASE_LEVEL >= 6:
                        queue_gating(bc, e, pl)
                    # next chunk's transposes/gate/LN at the expert boundary:
                    # the PE stream is already broken here by the mm2 batch,
                    # so the transpose mode-switch penalty is amortized
                    if bc + 1 < NBC:
                        # tiles front-loaded (0,1@e0 1@e1 1@e2) so tile 3's
                        # normedT copy lands well before tg_gates reads it
                        for t in ([0, 1], [2], [3], [])[e]:
                            tg_tile_T(bc + 1, t)
                        if e == E - 1:
                            tg_gates(bc + 1)
                            if bc + 2 < NBC:
                                prep(bc + 2)

            prep_dma(0)  # first: chunk-0 x tiles lead the DMA queue
            prep(0)      # chunk-0 LN chain leads the DVE queue
            # memsets after prep so the DVE reaches LN stats immediately
            nc.vector.memset(accT3[:], 0.0)
            nc.sync.dma_start(gwsb[:], gw[:])
            nc.sync.dma_start(gbsb[:], gb[:])
            nc.sync.dma_start(b2sb[:], b2[:])
            nc.sync.dma_start(b1sb[:], b1[:])
            nc.sync.dma_start(w2sb[:], w2[:])
            # w1 preload on the SAME sync queue, behind chunk-0's x tiles:
            # single-queue FIFO gives the exact consumption order.
            # H-quarter pieces let expert 0's first mm1 groups start after
            # 1MB instead of 4MB.  Chunk-1's x tiles slot in right after
            # expert 0's w1 so the chunk-1 transposes (which now run at
            # expert boundaries of chunk 0) never wait on the 16MB w1 bulk.
            HQ = H // 4
            for e in range(E):
                for q in range(4):
                    nc.sync.dma_start(w1sb[:, e, :, q * HQ:(q + 1) * HQ],
                                      w1[e, :, :, q * HQ:(q + 1) * HQ])
                if e == 0:
                    prep_dma(1)
            if PHASE_LEVEL >= 5:
                transpose_gate(0)
                prep(1)
                for bc in range(NBC):
                    experts(bc)
                flush_gating()
            else:
                for bc in range(NBC):
                    transpose_gate(bc)
                    if bc + 1 < NBC:
                        prep(bc + 1)
                nc.sync.dma_start(y[0:C, :], accT3[0:C, :])

    nc.finalize()
    return nc


def _fold_inputs(inputs):
    x = np.asarray(inputs["x"], np.float32)
    gg = np.asarray(inputs["gate_ln_g"], np.float32)
    gbeta = np.asarray(inputs["gate_ln_b"], np.float32)
    gw_ = np.asarray(inputs["gate_w"], np.float32)
    gbias = np.asarray(inputs["gate_b"], np.float32)
    eg = np.asarray(inputs["ex_ln_g"], np.float32)
    eb = np.asarray(inputs["ex_ln_b"], np.float32)
    w1_ = np.asarray(inputs["ex_w1"], np.float32)
    b1_ = np.asarray(inputs["ex_b1"], np.float32)
    w2_ = np.asarray(inputs["ex_w2"], np.float32)
    b2_ = np.asarray(inputs["ex_b2"], np.float32)

    # fold the (shared-normalize, per-head affine) LayerNorms into the
    # following linear layers: (n*g+b) @ W == n @ (g[:,None]*W) + b@W
    gwf = (gg[:, None] * gw_).astype(np.float32)                    # [D, E]
    gbf = (gbias + gbeta @ gw_).astype(np.float32)                  # [E]
    w1f = (eg[:, :, None] * w1_).astype(np.float32)                 # [E, D, H]
    b1f = (b1_ + np.einsum("ed,edh->eh", eb, w1_)).astype(np.float32)

    bf16 = ml_dtypes.bfloat16
    gw_dev = np.ascontiguousarray(
        gwf.reshape(KC, 128, E).transpose(1, 0, 2)).astype(bf16)
    gb_dev = np.ascontiguousarray(np.tile(gbf.reshape(1, E), (128, 1))).astype(bf16)
    w1_dev = np.ascontiguousarray(
        w1f.reshape(E, KC, 128, H).transpose(0, 2, 1, 3)).astype(bf16)
    b1_dev = np.ascontiguousarray(
        b1f.reshape(E, NHC, 128).transpose(2, 0, 1))                # [128, E, NHC]
    w2_dev = np.ascontiguousarray(
        w2_.reshape(E, NHC, 128, C).transpose(2, 0, 1, 3).reshape(
            128, E, NHC * C)).astype(bf16)
    b2_dev = np.ascontiguousarray(b2_.T)
    weights = dict(gw=gw_dev, gb=gb_dev, w1=w1_dev, b1=b1_dev,
                   w2=w2_dev, b2=b2_dev)
    return x, weights


def _get_nc():
    if "nc" not in _NC_CACHE:
        _NC_CACHE["nc"] = _build_nc()
    return _NC_CACHE["nc"]


def _in_maps(inputs):
    x, weights = _fold_inputs(inputs)
    maps = []
    for c in range(N_CORES):
        m = dict(weights)
        m["x"] = np.ascontiguousarray(x[c * BS:(c + 1) * BS]).astype(
            ml_dtypes.bfloat16)
        maps.append(m)
    return maps


def kernel(**inputs) -> np.ndarray:
    nc = _get_nc()
    maps = _in_maps(inputs)
    try:
        res = run_bass_kernel_spmd(nc, maps, list(range(N_CORES))).results
    except Exception:  # transient device error: retry once
        res = run_bass_kernel_spmd(nc, maps, list(range(N_CORES))).results
    out = np.empty((B, C), np.float32)
    for c in range(N_CORES):
        yc = res[c]["y"]
        out[c * BS:(c + 1) * BS] = (yc[0:C] + yc[C:2 * C] + yc[2 * C:3 * C]).T
    return out

